# revision 4
# baseline (speedup 1.0000x reference)
"""ROIPool (torchvision semantics) on 8 Trainium2 NeuronCores.

Strategy (branch-free SPMD):
  - core (ch, q), ch in {0,1}, q in {0..3}: channel-half ch, ROI-quarter q.
  - Host computes, per ROI, the exact pooling-bin pixel lists (replicating
    the reference's fp32 bin arithmetic), pads every bin of a ROI-slot to a
    uniform capacity L_s, and encodes them as int16 gather indices into the
    [128, 8192] (c, b*4096+h*64+w) on-chip feature tile.
  - Device: GpSimd ap_gather materializes the padded pixel streams; VectorE
    does one grouped max-reduce per slot ([128, 49, L_s] -> [128, 49]); a
    mask multiply zeroes empty bins; DMA out.
  - All ROI geometry lives in input data (indices/mask), so every core runs
    the identical program; only the per-slot capacity schedule is baked at
    trace time (compiled per distinct schedule, cached).
"""

import sys

sys.path.insert(0, "/opt/trn_rl_repo")

import numpy as np

N, C, H, W = 2, 256, 64, 64
K = 128
PH = PW = 7
SPATIAL_SCALE = np.float32(0.0625)
N_CORES = 8
N_Q = 4                # ROI quarters
SLOTS = K // N_Q       # 32 slots per core
FEAT_ELEMS = N * H * W  # 8192
GROUP_TARGET = 4096    # target gather stream els per ap_gather call

_prog_cache = {}


def _bin_edges(start, length, pooled, size):
    """Replicate reference _bin_masks start/end computation in fp32."""
    bin_sz = length.astype(np.float32) / np.float32(pooled)      # [K]
    j = np.arange(pooled, dtype=np.float32)
    s = np.floor(j[None, :] * bin_sz[:, None]).astype(np.int32) + start[:, None]
    e = np.ceil((j[None, :] + np.float32(1.0)) * bin_sz[:, None]).astype(np.int32) + start[:, None]
    return np.clip(s, 0, size), np.clip(e, 0, size)


def _roi_geometry(rois):
    b = rois[:, 0].astype(np.int32)
    coords = np.round(rois[:, 1:].astype(np.float32) * SPATIAL_SCALE).astype(np.int32)
    x1, y1, x2, y2 = coords[:, 0], coords[:, 1], coords[:, 2], coords[:, 3]
    rw = np.maximum(x2 - x1 + 1, 1)
    rh = np.maximum(y2 - y1 + 1, 1)
    sx, ex = _bin_edges(x1, rw, PW, W)   # [K, 7]
    sy, ey = _bin_edges(y1, rh, PH, H)   # [K, 7]
    return b, sx, ex, sy, ey


def _build_schedule(b, sx, ex, sy, ey):
    """Assign ROIs to (q, slot); compute per-slot capacities (shared by all
    cores) and per-q index streams / masks / gather grouping."""
    bw = ex - sx                                    # [K, 7]
    bh = ey - sy
    area = bh[:, :, None] * bw[:, None, :]          # [K, 7, 7]
    Lk = area.reshape(K, -1).max(axis=1)            # max bin area per ROI
    Lp = np.maximum(4, ((Lk + 3) // 4) * 4)         # mult of 4, >= 4

    order = np.argsort(-Lp, kind="stable")          # ranks: descending capacity
    # rank r -> core-quarter q = r % 4, slot s = r // 4
    slot_caps = tuple(int(Lp[order[s * N_Q]]) for s in range(SLOTS))

    # gather groups: consecutive slots, stream target ~GROUP_TARGET els
    groups = []
    cur, cur_sz = [], 0
    for s in range(SLOTS):
        sz = 49 * slot_caps[s]
        if cur and cur_sz + sz > GROUP_TARGET:
            groups.append(tuple(cur))
            cur, cur_sz = [], 0
        cur.append(s)
        cur_sz += sz
    if cur:
        groups.append(tuple(cur))

    # per-group padded num_idxs (mult of 32 keeps idx column offsets 4B-aligned)
    group_n = []
    for grp in groups:
        n = sum(49 * slot_caps[s] for s in grp)
        group_n.append(((n + 31) // 32) * 32)
    IW = sum(n // 16 for n in group_n)              # idx tensor columns

    # per-q index streams and masks
    idx_arrs = np.zeros((N_Q, 128, IW), dtype=np.int16)
    mask_arrs = np.ones((N_Q, SLOTS * 49), dtype=np.float32)
    assign = {}                                     # roi k -> (q, s)
    for r in range(K):
        k = int(order[r])
        assign[k] = (r % N_Q, r // N_Q)

    for q in range(N_Q):
        roi_of_slot = {}
        for k, (qq, s) in assign.items():
            if qq == q:
                roi_of_slot[s] = k
        col = 0
        for gi, grp in enumerate(groups):
            stream = []
            for s in grp:
                L = slot_caps[s]
                k = roi_of_slot.get(s)
                if k is None:
                    stream.extend([0] * (49 * L))
                    mask_arrs[q, s * 49:(s + 1) * 49] = 0.0
                    continue
                base = int(b[k]) * (H * W)
                for i in range(PH):
                    ys, ye = int(sy[k, i]), int(ey[k, i])
                    for j in range(PW):
                        xs, xe = int(sx[k, j]), int(ex[k, j])
                        cnt = (ye - ys) * (xe - xs)
                        if cnt <= 0:
                            stream.extend([0] * L)
                            mask_arrs[q, s * 49 + i * 7 + j] = 0.0
                            continue
                        px = [base + h * W + w
                              for h in range(ys, ye) for w in range(xs, xe)]
                        px.extend([px[0]] * (L - cnt))
                        stream.extend(px)
            n_pad = group_n[gi]
            stream.extend([0] * (n_pad - len(stream)))
            arr = np.asarray(stream, dtype=np.int16).reshape(n_pad // 16, 16).T  # [16, n/16]
            ncols = n_pad // 16
            for g8 in range(8):
                idx_arrs[q, g8 * 16:(g8 + 1) * 16, col:col + ncols] = arr
            col += ncols
        assert col == IW

    return slot_caps, tuple(groups), tuple(group_n), IW, idx_arrs, mask_arrs, assign


def _fix_drain_waits(nc, mybir):
    """Codegen allows only 1 sync-wait on ctrl (Drain) instructions; split
    extra waits onto injected preceding drains."""
    MAXW = 1
    for bb in nc.m.functions[0].blocks:
        newlist = []
        for inst in bb.instructions:
            w = list(inst.sync_info.on_wait) if inst.sync_info and inst.sync_info.on_wait else []
            if len(w) > MAXW and type(inst).__name__ == "InstDrain":
                kk = 0
                while len(w) - kk > MAXW:
                    chunk = w[kk:kk + MAXW]
                    kk += MAXW
                    d = mybir.InstDrain(name=f"{inst.name}-wsplit{kk}", ins=[], outs=[])
                    d.engine = inst.engine
                    d.sync_info = mybir.SyncInfo(on_wait=chunk, on_update=[])
                    nc.register_instruction(d)
                    newlist.append(d)
                inst.sync_info.on_wait = w[kk:]
            newlist.append(inst)
        bb.instructions = newlist


def _build_program(slot_caps, groups, group_n, IW, repeat=1):
    import concourse.tile as tile
    from concourse import mybir, bacc

    S49 = SLOTS * 49
    nc = bacc.Bacc("TRN2", target_bir_lowering=False, debug=False)
    feat_d = nc.dram_tensor("feat", [128, FEAT_ELEMS], mybir.dt.float32, kind="ExternalInput").ap()
    idx_d = nc.dram_tensor("idxs", [128, IW], mybir.dt.int16, kind="ExternalInput").ap()
    mask_d = nc.dram_tensor("mask", [128, S49], mybir.dt.float32, kind="ExternalInput").ap()
    out_d = nc.dram_tensor("out", [128, S49], mybir.dt.float32, kind="ExternalOutput").ap()

    with tile.TileContext(nc) as tc:
        import contextlib
        with contextlib.ExitStack() as ctx:
            fpool = ctx.enter_context(tc.tile_pool(name="fpool", bufs=1))
            gpool = ctx.enter_context(tc.tile_pool(name="gpool", bufs=3))
            opool = ctx.enter_context(tc.tile_pool(name="opool", bufs=2))

            feat_t = fpool.tile([128, FEAT_ELEMS], mybir.dt.float32, tag="feat")
            nc.sync.dma_start(feat_t[:], feat_d[:])
            idx_t = fpool.tile([128, IW], mybir.dt.int16, tag="idx")
            nc.sync.dma_start(idx_t[:], idx_d[:])
            mask_t = fpool.tile([128, S49], mybir.dt.float32, tag="mask")
            nc.sync.dma_start(mask_t[:], mask_d[:])

            def body():
                out_t = opool.tile([128, S49], mybir.dt.float32, tag="out")
                col = 0
                for gi, grp in enumerate(groups):
                    n_pad = group_n[gi]
                    g_t = gpool.tile([128, n_pad], mybir.dt.float32, tag="g")
                    nc.gpsimd.ap_gather(
                        g_t[:, :n_pad],
                        feat_t[:, :],
                        idx_t[:, col:col + n_pad // 16],
                        channels=128,
                        num_elems=FEAT_ELEMS,
                        d=1,
                        num_idxs=n_pad,
                    )
                    col += n_pad // 16
                    off = 0
                    for s in grp:
                        L = slot_caps[s]
                        seg = g_t[:, off:off + 49 * L].rearrange("p (k u) -> p k u", u=L)
                        nc.vector.reduce_max(
                            out_t[:, s * 49:(s + 1) * 49], seg,
                            axis=mybir.AxisListType.X,
                        )
                        off += 49 * L
                out2_t = opool.tile([128, S49], mybir.dt.float32, tag="out2")
                nc.vector.tensor_mul(out2_t[:], out_t[:], mask_t[:])
                nc.sync.dma_start(out_d[:], out2_t[:])

            if repeat == 1:
                body()
            else:
                with tc.For_i(0, repeat, 1):
                    body()

    nc.finalize()
    return nc


def _get_program(key):
    if key not in _prog_cache:
        slot_caps, groups, group_n, IW, repeat = key
        _prog_cache[key] = _build_program(slot_caps, groups, group_n, IW, repeat)
    return _prog_cache[key]


def _run(feat_halves, idx_arrs, mask_arrs, key):
    from concourse.bass_utils import run_bass_kernel_spmd

    nc = _get_program(key)
    in_maps = []
    for core in range(N_CORES):
        ch, q = core // N_Q, core % N_Q
        in_maps.append({
            "feat": feat_halves[ch],
            "idxs": np.ascontiguousarray(idx_arrs[q]),
            "mask": np.ascontiguousarray(
                np.broadcast_to(mask_arrs[q][None, :], (128, mask_arrs[q].shape[0]))),
        })
    res = run_bass_kernel_spmd(nc, in_maps, list(range(N_CORES)))
    return res


def kernel(input, rois, _repeat=1, _return_res=False):
    input = np.asarray(input, dtype=np.float32)
    rois = np.asarray(rois, dtype=np.float32)

    b, sx, ex, sy, ey = _roi_geometry(rois)
    slot_caps, groups, group_n, IW, idx_arrs, mask_arrs, assign = _build_schedule(
        b, sx, ex, sy, ey)

    # per-channel-half feature tiles [128, 8192] laid out (c, b*4096 + h*64 + w)
    feat_halves = [
        np.ascontiguousarray(
            input[:, ch * 128:(ch + 1) * 128].transpose(1, 0, 2, 3).reshape(128, FEAT_ELEMS))
        for ch in range(2)
    ]

    key = (slot_caps, groups, group_n, IW, _repeat)
    res = _run(feat_halves, idx_arrs, mask_arrs, key)

    out = np.empty((K, C, PH, PW), dtype=np.float32)
    for k in range(K):
        q, s = assign[k]
        for ch in range(2):
            core = ch * N_Q + q
            blk = res.results[core]["out"][:, s * 49:(s + 1) * 49]
            out[k, ch * 128:(ch + 1) * 128] = blk.reshape(128, PH, PW)
    if _return_res:
        return out, res
    return out
